# revision 5
# baseline (speedup 1.0000x reference)
"""CapsuleLayer (single routing iteration) Trainium2 kernel.

Math (per batch element b of x: (B=64, NU=32, IC=256, US=128) fp32):
  - torch-style reshape of x[b] to (IC, NU, US): row i of the flat
    (256, 4096) view is x[b].flat[i*4096:(i+1)*4096].
  - s[j]   = (1/256) * sum_i flat[i, j]          (j = n*128+u, 4096 outputs)
  - msq[n] = sum_u s[n,u]^2
  - out[n,u] = msq/(1+msq) * s[n,u]/(sqrt(msq)+1e-5)
             ~ s[n,u] * sqrt(msq)/(1+msq)        (1e-5 dropped: ~1.4e-5 rel)

Sharding: pure batch data-parallel over 8 NeuronCores (8 batches/core).

Per-core pipeline (memory-bound; ~32 MiB HBM reads per core; a single
HWDGE queue saturates ~425-430 GB/s, so the two rings are used to hide
per-DMA completion bubbles, not for bandwidth):
  - per batch: h0 half (2 MiB) on the SP ring, h1 half on the ACT ring.
    Batch 0 is split in half-chunks (pipe fill), batch 7 in 16x 512-col
    chunks alternating rings so the PE trails the last byte by ~1 matmul.
  - 16 float32r matmuls per batch reduce ic via the PE into one PSUM
    (8, 512) accumulation group (lhsT column k holds 1/256).
  - squash runs on DVE (4x tensor_tensor_reduce -> msq, one scalar Sqrt,
    scalar_tensor_tensor + broadcast tensor_tensor), software-pipelined
    two batches behind the loads so the scalar engine's HWDGE ring is
    never blocked behind squash work (the v1 kernel lost ~5 us to that).
  - all 8 batch outputs accumulate in one SBUF tile; ONE 128 KiB store
    at the end (v1 used 8 SWDGE stores + drains on the critical tail).

float32r streams fp32 through the PE in one pass by rounding the operands
to ~12 mantissa bits (same 4-byte encoding, so the host feeds plain fp32
bytes); weights are powers of two (exact), so output rel-err is ~1e-4.
"""

import numpy as np

import concourse.bass as bass
import concourse.bacc as bacc
import concourse.mybir as mybir
import concourse.tile as tile
from concourse.bass_utils import run_bass_kernel_spmd

B, NU, IC, US = 64, 32, 256, 128
N_CORES = 8
PB = B // N_CORES            # batches per core
F = NU * US                  # 4096 outputs per batch
HALVES = IC // 128           # 2 partition-halves of the ic axis
NBANK = F // 512             # 8 matmul chunks (one PSUM partition row each)
NQ = F // NBANK // 128       # 4 u-groups per PSUM partition row

# True: factor applied via DVE divide (2 ops); False: reciprocal chain
# (4 ops). divide is NOT a valid DVE ALU op on TRN2 (codegen rejects it),
# so only False works.
SQUASH_DIV = False


def build_bass(pb=PB, squash_div=SQUASH_DIV):
    PB = pb
    nc = bacc.Bacc("TRN2", target_bir_lowering=False, debug=False)

    mm_dt = mybir.dt.float32r

    # float32r shares the fp32 byte encoding (it is fp32 with the mantissa
    # rounded to ~12 bits by the PE), so the host feeds plain fp32 bytes.
    x = nc.dram_tensor("x", [PB, HALVES, 128, F], mm_dt,
                       kind="ExternalInput")
    w = nc.dram_tensor("w", [128, NBANK, NBANK], mm_dt,
                       kind="ExternalInput")
    y = nc.dram_tensor("y", [PB, NBANK, 512], mybir.dt.float32,
                       kind="ExternalOutput")

    with tile.TileContext(nc) as tc:
        with (
            tc.tile_pool(name="const", bufs=1) as const_pool,
            tc.tile_pool(name="acc", bufs=8) as acc_pool,
            tc.tile_pool(name="psum", bufs=8, space="PSUM") as psum_pool,
            tc.tile_pool(name="sq", bufs=2) as sq_pool,
            tc.tile_pool(name="stats", bufs=8) as stats_pool,
            tc.tile_pool(name="outp", bufs=1) as out_pool,
        ):
            # Selection weights: sel[:, k, j] = 1/256 iff j == k.
            # (loaded from DRAM - memset can't emit float32r; SWDGE keeps
            # it off the two load rings)
            sel = const_pool.tile([128, NBANK, NBANK], mm_dt)
            nc.gpsimd.dma_start(out=sel[:], in_=w[:])

            # All 8 batches' squashed outputs land here; one store at the end.
            outt = out_pool.tile([NBANK, PB, 512], mybir.dt.float32)

            state = {}

            def loads(b):
                th = []
                for h in range(HALVES):
                    t = acc_pool.tile([128, F], mm_dt, tag="acc")
                    th.append(t)
                rings = (nc.sync, nc.scalar)
                if b == 0:
                    # Pipe fill: two 1 MiB chunks per half.
                    for h in range(HALVES):
                        for c in range(2):
                            rings[h].dma_start(
                                out=th[h][:, c * (F // 2):(c + 1) * (F // 2)],
                                in_=x[b, h, :, c * (F // 2):(c + 1) * (F // 2)])
                elif b == PB - 1:
                    # Fine tail: 16x 512-col chunks alternating rings so the
                    # final matmul waits on only 256 KiB of residual data.
                    for h in range(HALVES):
                        for k in range(NBANK):
                            rings[(h * NBANK + k) % 2].dma_start(
                                out=th[h][:, k * 512:(k + 1) * 512],
                                in_=x[b, h, :, k * 512:(k + 1) * 512])
                else:
                    for h in range(HALVES):
                        rings[h].dma_start(out=th[h][:], in_=x[b, h])
                return th

            def mms(b, th):
                ps = psum_pool.tile([NBANK, 512], mybir.dt.float32, tag="ps")
                for h in range(HALVES):
                    for k in range(NBANK):
                        nc.tensor.matmul(
                            ps[:, :],
                            sel[:, k, :],
                            th[h][:, k * 512:(k + 1) * 512],
                            start=(h == 0 and k == 0),
                            stop=(h == HALVES - 1 and k == NBANK - 1),
                        )
                return ps

            def finish(b):
                ps = state.pop(b)
                # One Square over the whole (8, 512) (no accumulator-read
                # ping-pong), then one DVE reduce for the per-group sums.
                # Deferred 2 batches behind the loads so this never sits
                # between two load triggers in the scalar engine's FIFO.
                sq = sq_pool.tile([NBANK, 512], mybir.dt.float32, tag="sq")
                nc.scalar.activation(out=sq[:], in_=ps[:],
                                     func=mybir.ActivationFunctionType.Square)
                msq = stats_pool.tile([NBANK, NQ], mybir.dt.float32,
                                      tag="msq")
                nc.vector.tensor_reduce(
                    out=msq[:],
                    in_=sq[:].rearrange("p (q u) -> p q u", q=NQ),
                    axis=mybir.AxisListType.X,
                    op=mybir.AluOpType.add)
                mag = stats_pool.tile([NBANK, NQ], mybir.dt.float32, tag="mag")
                nc.scalar.activation(out=mag[:], in_=msq[:],
                                     func=mybir.ActivationFunctionType.Sqrt)
                od = outt[:, b, :].rearrange("p (q u) -> p q u", q=NQ)
                psr = ps[:].rearrange("p (q u) -> p q u", q=NQ)
                if squash_div:
                    # fi = (1 + msq) / mag ; out = s / fi
                    fi = stats_pool.tile([NBANK, NQ], mybir.dt.float32,
                                         tag="fi")
                    nc.vector.scalar_tensor_tensor(
                        out=fi[:], in0=msq[:], scalar=1.0, in1=mag[:],
                        op0=mybir.AluOpType.add, op1=mybir.AluOpType.divide)
                    fap = fi[:]
                    fb = bass.AP(tensor=fap.tensor, offset=fap.offset,
                                 ap=[fap.ap[0], fap.ap[1], [0, 128]])
                    nc.vector.tensor_tensor(od, psr, fb,
                                            mybir.AluOpType.divide)
                else:
                    # fac = mag * 1/(1 + msq) ; out = s * fac
                    t1 = stats_pool.tile([NBANK, NQ], mybir.dt.float32,
                                         tag="t1")
                    nc.vector.tensor_scalar_add(t1[:], msq[:], 1.0)
                    rec = stats_pool.tile([NBANK, NQ], mybir.dt.float32,
                                          tag="rec")
                    nc.vector.reciprocal(rec[:], t1[:])
                    fac = stats_pool.tile([NBANK, NQ], mybir.dt.float32,
                                          tag="fac")
                    nc.vector.tensor_mul(fac[:], mag[:], rec[:])
                    fap = fac[:]
                    fb = bass.AP(tensor=fap.tensor, offset=fap.offset,
                                 ap=[fap.ap[0], fap.ap[1], [0, 128]])
                    nc.vector.tensor_tensor(od, psr, fb,
                                            mybir.AluOpType.mult)

            for b in range(PB):
                th = loads(b)
                if b == PB - 1:
                    finish(PB - 3)
                    finish(PB - 2)
                elif b >= 2:
                    finish(b - 2)
                state[b] = mms(b, th)
            finish(PB - 1)

            # One 128 KiB store for all batches (SP ring is idle by now).
            nc.sync.dma_start(out=y[:].rearrange("b k c -> k b c"),
                              in_=outt[:])

    nc.compile()
    return nc


_NC_CACHE = {}


def _get_nc():
    if "nc" not in _NC_CACHE:
        _NC_CACHE["nc"] = build_bass()
    return _NC_CACHE["nc"]


def kernel(x, **run_kwargs):
    x = np.ascontiguousarray(np.asarray(x, dtype=np.float32))
    assert x.shape == (B, NU, IC, US), x.shape

    nc = _get_nc()
    xs = x.reshape(N_CORES, PB, HALVES, 128, F)
    w = np.zeros((128, NBANK, NBANK), dtype=np.float32)
    for k in range(NBANK):
        w[:, k, k] = 1.0 / IC
    in_maps = [{"x": np.ascontiguousarray(xs[c]), "w": w}
               for c in range(N_CORES)]
    res = run_bass_kernel_spmd(nc, in_maps, core_ids=list(range(N_CORES)),
                               **run_kwargs)
    out = np.stack([r["y"] for r in res.results], axis=0)  # (8, PB, 8, 512)
    out = out.reshape(B, NU, US, 1)
    if run_kwargs:
        kernel.last_results = res
    return out


# revision 7
# speedup vs baseline: 1.0314x; 1.0314x over previous
"""CapsuleLayer (single routing iteration) Trainium2 kernel.

Math (per batch element b of x: (B=64, NU=32, IC=256, US=128) fp32):
  - torch-style reshape of x[b] to (IC, NU, US): row i of the flat
    (256, 4096) view is x[b].flat[i*4096:(i+1)*4096].
  - s[j]   = (1/256) * sum_i flat[i, j]          (j = n*128+u, 4096 outputs)
  - msq[n] = sum_u s[n,u]^2
  - out[n,u] = msq/(1+msq) * s[n,u]/(sqrt(msq)+1e-5)
             ~ s[n,u] * sqrt(msq)/(1+msq)        (1e-5 dropped: ~1.4e-5 rel)

Sharding: pure batch data-parallel over 8 NeuronCores (8 batches/core).

Per-core pipeline (memory-bound; ~32 MiB HBM reads per core; one HWDGE
queue alone saturates ~425-430 GB/s, so the two rings exist to hide
per-DMA completion bubbles, not for bandwidth):
  - per batch: h0 half (2 MiB) on the SP ring, h1 half on the ACT ring;
    batch 7 in 8x 1 MiB chunks alternating rings so the PE trails the
    final byte by only ~2 matmuls.
  - 16 float32r matmuls per batch reduce ic via the PE into one PSUM
    (8, 512) accumulation group (lhsT column k holds 1/256).
  - squash: one scalar Square (8,512) -> DVE tensor_reduce -> scalar
    Sqrt -> DVE add/recip/mul -> DVE broadcast multiply. Deferred ONE
    batch behind the loads so squash work never sits between two load
    triggers in an engine's FIFO (HWDGE dispatch is in-order per
    engine; the v1 kernel lost ~5 us of HBM time to that).
  - all outputs accumulate in one SBUF tile; ONE contiguous 128 KiB
    store at the end (8x 16 KiB descriptors).

float32r streams fp32 through the PE in one pass by rounding operands to
~12 mantissa bits (same 4-byte encoding: the host feeds plain fp32
bytes); weights are powers of two (exact), so output rel-err is ~1e-4.
"""

import numpy as np

import concourse.bass as bass
import concourse.bacc as bacc
import concourse.mybir as mybir
import concourse.tile as tile
from concourse.bass_utils import run_bass_kernel_spmd

B, NU, IC, US = 64, 32, 256, 128
N_CORES = 8
PB = B // N_CORES            # batches per core
F = NU * US                  # 4096 outputs per batch
HALVES = IC // 128           # 2 partition-halves of the ic axis
NBANK = F // 512             # 8 matmul chunks (one PSUM partition row each)
NQ = F // NBANK // 128       # 4 u-groups per PSUM partition row
TC = 4                       # tail chunks per half for the last batch


def build_bass(pb=PB):
    PB = pb
    nc = bacc.Bacc("TRN2", target_bir_lowering=False, debug=False)

    mm_dt = mybir.dt.float32r

    x = nc.dram_tensor("x", [PB, HALVES, 128, F], mm_dt,
                       kind="ExternalInput")
    w = nc.dram_tensor("w", [128, NBANK, NBANK], mm_dt,
                       kind="ExternalInput")
    y = nc.dram_tensor("y", [NBANK, PB, 512], mybir.dt.float32,
                       kind="ExternalOutput")

    with tile.TileContext(nc) as tc:
        with (
            tc.tile_pool(name="const", bufs=1) as const_pool,
            tc.tile_pool(name="acc", bufs=6) as acc_pool,
            tc.tile_pool(name="psum", bufs=4, space="PSUM") as psum_pool,
            tc.tile_pool(name="sq", bufs=2) as sq_pool,
            tc.tile_pool(name="stats", bufs=4) as stats_pool,
            tc.tile_pool(name="outp", bufs=1) as out_pool,
        ):
            # Selection weights: sel[:, k, j] = 1/256 iff j == k.
            # (loaded from DRAM - memset can't emit float32r; SWDGE keeps
            # it off the two load rings)
            sel = const_pool.tile([128, NBANK, NBANK], mm_dt)
            nc.gpsimd.dma_start(out=sel[:], in_=w[:])

            # All 8 batches' squashed outputs land here; one store at the end.
            outt = out_pool.tile([NBANK, PB, 512], mybir.dt.float32)

            state = {}
            rings = (nc.sync, nc.scalar)

            def load_full(b):
                th = [acc_pool.tile([128, F], mm_dt, tag="acc",
                                    name=f"t{b}h{h}")
                      for h in range(HALVES)]
                for h in range(HALVES):
                    rings[h].dma_start(out=th[h][:], in_=x[b, h])
                return th

            def load_tail_chunks(b, th, cs):
                fc = F // TC
                for c in cs:
                    h, k = c // TC, c % TC
                    rings[c % 2].dma_start(
                        out=th[h][:, k * fc:(k + 1) * fc],
                        in_=x[b, h, :, k * fc:(k + 1) * fc])

            def mms(b, th):
                ps = psum_pool.tile([NBANK, 512], mybir.dt.float32, tag="ps")
                for h in range(HALVES):
                    for k in range(NBANK):
                        nc.tensor.matmul(
                            ps[:, :],
                            sel[:, k, :],
                            th[h][:, k * 512:(k + 1) * 512],
                            start=(h == 0 and k == 0),
                            stop=(h == HALVES - 1 and k == NBANK - 1),
                        )
                state[b] = ps

            def finish(b):
                ps = state.pop(b)
                sq = sq_pool.tile([NBANK, 512], mybir.dt.float32, tag="sq")
                nc.scalar.activation(out=sq[:], in_=ps[:],
                                     func=mybir.ActivationFunctionType.Square)
                msq = stats_pool.tile([NBANK, NQ], mybir.dt.float32,
                                      tag="msq")
                nc.vector.tensor_reduce(
                    out=msq[:],
                    in_=sq[:].rearrange("p (q u) -> p q u", q=NQ),
                    axis=mybir.AxisListType.X,
                    op=mybir.AluOpType.add)
                mag = stats_pool.tile([NBANK, NQ], mybir.dt.float32,
                                      tag="mag")
                nc.scalar.activation(out=mag[:], in_=msq[:],
                                     func=mybir.ActivationFunctionType.Sqrt)
                # fac = mag * 1/(1 + msq)  (== msq/((1+msq)sqrt(msq)))
                t1 = stats_pool.tile([NBANK, NQ], mybir.dt.float32, tag="t1")
                nc.vector.tensor_scalar_add(t1[:], msq[:], 1.0)
                rec = stats_pool.tile([NBANK, NQ], mybir.dt.float32,
                                      tag="rec")
                nc.vector.reciprocal(rec[:], t1[:])
                fac = stats_pool.tile([NBANK, NQ], mybir.dt.float32,
                                      tag="fac")
                nc.vector.tensor_mul(fac[:], mag[:], rec[:])
                fap = fac[:]
                fb = bass.AP(tensor=fap.tensor, offset=fap.offset,
                             ap=[fap.ap[0], fap.ap[1], [0, 128]])
                nc.vector.tensor_tensor(
                    outt[:, b, :].rearrange("p (q u) -> p q u", q=NQ),
                    ps[:].rearrange("p (q u) -> p q u", q=NQ),
                    fb, mybir.AluOpType.mult)

            for b in range(PB):
                if b < PB - 1:
                    th = load_full(b)
                    if b >= 1:
                        finish(b - 1)
                else:
                    th = [acc_pool.tile([128, F], mm_dt, tag="acc",
                                        name=f"t{b}h{h}")
                          for h in range(HALVES)]
                    load_tail_chunks(b, th, range(TC))
                    finish(b - 1)
                    load_tail_chunks(b, th, range(TC, HALVES * TC))
                mms(b, th)
            finish(PB - 1)

            # One contiguous 128 KiB store (8 x 16 KiB descriptors).
            nc.sync.dma_start(out=y[:], in_=outt[:])

    nc.compile()
    return nc


_NC_CACHE = {}


def _get_nc():
    if "nc" not in _NC_CACHE:
        _NC_CACHE["nc"] = build_bass()
    return _NC_CACHE["nc"]


def kernel(x, **run_kwargs):
    x = np.ascontiguousarray(np.asarray(x, dtype=np.float32))
    assert x.shape == (B, NU, IC, US), x.shape

    nc = _get_nc()
    xs = x.reshape(N_CORES, PB, HALVES, 128, F)
    w = np.zeros((128, NBANK, NBANK), dtype=np.float32)
    for k in range(NBANK):
        w[:, k, k] = 1.0 / IC
    in_maps = [{"x": np.ascontiguousarray(xs[c]), "w": w}
               for c in range(N_CORES)]
    res = run_bass_kernel_spmd(nc, in_maps, core_ids=list(range(N_CORES)),
                               **run_kwargs)
    # y is (NBANK, PB, 512) per core; batch-major reshape on the host.
    out = np.stack([r["y"].transpose(1, 0, 2) for r in res.results], axis=0)
    out = out.reshape(B, NU, US, 1)
    if run_kwargs:
        kernel.last_results = res
    return out


# revision 10
# speedup vs baseline: 1.0532x; 1.0211x over previous
"""CapsuleLayer (single routing iteration) Trainium2 kernel.

Math (per batch element b of x: (B=64, NU=32, IC=256, US=128) fp32):
  - torch-style reshape of x[b] to (IC, NU, US): row i of the flat
    (256, 4096) view is x[b].flat[i*4096:(i+1)*4096].
  - s[j]   = (1/256) * sum_i flat[i, j]          (j = n*128+u, 4096 outputs)
  - msq[n] = sum_u s[n,u]^2
  - out[n,u] = msq/(1+msq) * s[n,u]/(sqrt(msq)+1e-5)
             ~ s[n,u] * sqrt(msq)/(1+msq)        (1e-5 dropped: ~1.4e-5 rel)

Sharding: pure batch data-parallel over 8 NeuronCores (8 batches/core).

Per-core pipeline (memory-bound; ~32 MiB HBM reads per core; one HWDGE
queue alone saturates ~425-430 GB/s, so the two rings exist to hide
per-DMA completion bubbles, not for bandwidth):
  - per batch: h0 half (2 MiB) on the SP ring, h1 half on the ACT ring;
    batch 7 in 8x 1 MiB chunks alternating rings so the PE trails the
    final byte by only ~2 matmuls.
  - 16 float32r matmuls per batch reduce ic via the PE into one PSUM
    (8, 512) accumulation group (lhsT column k holds 1/256).
  - squash: one scalar Square (8,512) -> DVE tensor_reduce -> scalar
    Sqrt -> DVE add/recip/mul -> DVE broadcast multiply. Deferred ONE
    batch behind the loads so squash work never sits between two load
    triggers in an engine's FIFO (HWDGE dispatch is in-order per
    engine; the v1 kernel lost ~5 us of HBM time to that).
  - all outputs accumulate in one SBUF tile; ONE contiguous 128 KiB
    store at the end (8x 16 KiB descriptors).

float32r streams fp32 through the PE in one pass by rounding operands to
~12 mantissa bits (same 4-byte encoding: the host feeds plain fp32
bytes); weights are powers of two (exact), so output rel-err is ~1e-4.
"""

import numpy as np

import concourse.bass as bass
import concourse.bacc as bacc
import concourse.mybir as mybir
import concourse.tile as tile
from concourse.bass_utils import run_bass_kernel_spmd

B, NU, IC, US = 64, 32, 256, 128
N_CORES = 8
PB = B // N_CORES            # batches per core
F = NU * US                  # 4096 outputs per batch
HALVES = IC // 128           # 2 partition-halves of the ic axis
NBANK = F // 512             # 8 matmul chunks (one PSUM partition row each)
NQ = F // NBANK // 128       # 4 u-groups per PSUM partition row
TC = 8                       # tail chunks per half for the last batch


def build_bass(pb=PB):
    PB = pb
    nc = bacc.Bacc("TRN2", target_bir_lowering=False, debug=False)

    mm_dt = mybir.dt.float32r

    x = nc.dram_tensor("x", [PB, HALVES, 128, F], mm_dt,
                       kind="ExternalInput")
    w = nc.dram_tensor("w", [128, NBANK, NBANK], mm_dt,
                       kind="ExternalInput")
    y = nc.dram_tensor("y", [NBANK, PB, 512], mybir.dt.float32,
                       kind="ExternalOutput")

    with tile.TileContext(nc) as tc:
        with (
            tc.tile_pool(name="const", bufs=1) as const_pool,
            tc.tile_pool(name="acc", bufs=6) as acc_pool,
            tc.tile_pool(name="psum", bufs=4, space="PSUM") as psum_pool,
            tc.tile_pool(name="sq", bufs=2) as sq_pool,
            tc.tile_pool(name="stats", bufs=4) as stats_pool,
            tc.tile_pool(name="outp", bufs=1) as out_pool,
        ):
            # Selection weights: sel[:, k, j] = 1/256 iff j == k.
            # (loaded from DRAM - memset can't emit float32r; SWDGE keeps
            # it off the two load rings)
            sel = const_pool.tile([128, NBANK, NBANK], mm_dt)
            nc.gpsimd.dma_start(out=sel[:], in_=w[:])

            # All 8 batches' squashed outputs land here; one store at the end.
            outt = out_pool.tile([NBANK, PB, 512], mybir.dt.float32)

            state = {}
            rings = (nc.sync, nc.scalar)

            def load_full(b):
                th = [acc_pool.tile([128, F], mm_dt, tag="acc",
                                    name=f"t{b}h{h}")
                      for h in range(HALVES)]
                for h in range(HALVES):
                    rings[h].dma_start(out=th[h][:], in_=x[b, h])
                return th

            def load_tail_chunks(b, th, cs):
                fc = F // TC
                for c in cs:
                    h, k = c // TC, c % TC
                    rings[c % 2].dma_start(
                        out=th[h][:, k * fc:(k + 1) * fc],
                        in_=x[b, h, :, k * fc:(k + 1) * fc])

            def mms(b, th):
                ps = psum_pool.tile([NBANK, 512], mybir.dt.float32, tag="ps")
                for h in range(HALVES):
                    for k in range(NBANK):
                        nc.tensor.matmul(
                            ps[:, :],
                            sel[:, k, :],
                            th[h][:, k * 512:(k + 1) * 512],
                            start=(h == 0 and k == 0),
                            stop=(h == HALVES - 1 and k == NBANK - 1),
                        )
                state[b] = ps

            def squash(ps, nr, out_ap, tg):
                # squash a (nr, 512) PSUM block into out_ap (same partitions)
                sq = sq_pool.tile([nr, 512], mybir.dt.float32, tag="sq" + tg,
                                  name="sq" + tg)
                nc.scalar.activation(out=sq[:], in_=ps[:],
                                     func=mybir.ActivationFunctionType.Square)
                msq = stats_pool.tile([nr, NQ], mybir.dt.float32,
                                      tag="msq" + tg, name="msq" + tg)
                nc.vector.tensor_reduce(
                    out=msq[:],
                    in_=sq[:].rearrange("p (q u) -> p q u", q=NQ),
                    axis=mybir.AxisListType.X,
                    op=mybir.AluOpType.add)
                mag = stats_pool.tile([nr, NQ], mybir.dt.float32,
                                      tag="mag" + tg, name="mag" + tg)
                nc.scalar.activation(out=mag[:], in_=msq[:],
                                     func=mybir.ActivationFunctionType.Sqrt)
                # fac = mag * 1/(1 + msq)  (== msq/((1+msq)sqrt(msq)))
                t1 = stats_pool.tile([nr, NQ], mybir.dt.float32,
                                     tag="t1" + tg, name="t1" + tg)
                nc.vector.tensor_scalar_add(t1[:], msq[:], 1.0)
                rec = stats_pool.tile([nr, NQ], mybir.dt.float32,
                                      tag="rec" + tg, name="rec" + tg)
                nc.vector.reciprocal(rec[:], t1[:])
                fac = stats_pool.tile([nr, NQ], mybir.dt.float32,
                                      tag="fac" + tg, name="fac" + tg)
                nc.vector.tensor_mul(fac[:], mag[:], rec[:])
                fap = fac[:]
                fb = bass.AP(tensor=fap.tensor, offset=fap.offset,
                             ap=[fap.ap[0], fap.ap[1], [0, 128]])
                nc.vector.tensor_tensor(
                    out_ap.rearrange("p (q u) -> p q u", q=NQ),
                    ps[:].rearrange("p (q u) -> p q u", q=NQ),
                    fb, mybir.AluOpType.mult)

            def finish(b):
                squash(state.pop(b), NBANK, outt[:, b, :], "")

            for b in range(PB - 1):
                th = load_full(b)
                if b >= 1:
                    finish(b - 1)
                mms(b, th)

            # Last batch: fine-grained chunks on both rings; PSUM split in
            # two row-groups so half the squash hides under the last loads.
            b = PB - 1
            th = [acc_pool.tile([128, F], mm_dt, tag="acc", name=f"t{b}h{h}")
                  for h in range(HALVES)]
            load_tail_chunks(b, th, range(TC))
            finish(b - 1)
            load_tail_chunks(b, th, range(TC, HALVES * TC))
            # Batches 0-6 stored early: the HBM-write receipt hides under
            # the remaining loads instead of the kernel tail.
            nc.sync.dma_start(out=y[:, :PB - 1], in_=outt[:, :PB - 1])

            psA = psum_pool.tile([4, 512], mybir.dt.float32, tag="psA",
                                 bufs=1)
            psB = psum_pool.tile([4, 512], mybir.dt.float32, tag="psB",
                                 bufs=1)
            for h in range(HALVES):
                for k in range(NBANK):
                    ps, off = (psA, 0) if k < 4 else (psB, 4)
                    nc.tensor.matmul(
                        ps[:, :],
                        sel[:, k, off:off + 4],
                        th[h][:, k * 512:(k + 1) * 512],
                        start=(h == 0 and k % 4 == 0),
                        stop=(h == HALVES - 1 and k % 4 == 3),
                    )
            # Group A (chunks 0-3) accumulation closes ~2.5 us before group
            # B: its squash+store run while B's last chunks still stream.
            squash(psA, 4, outt[:4, b, :], "A")
            nc.sync.dma_start(out=y[:4, b], in_=outt[:4, b])
            outb = out_pool.tile([4, 512], mybir.dt.float32)
            squash(psB, 4, outb[:], "B")
            nc.scalar.dma_start(out=y[4:, b], in_=outb[:])

    nc.compile()
    return nc


_NC_CACHE = {}


def _get_nc():
    if "nc" not in _NC_CACHE:
        _NC_CACHE["nc"] = build_bass()
    return _NC_CACHE["nc"]


def kernel(x, **run_kwargs):
    x = np.ascontiguousarray(np.asarray(x, dtype=np.float32))
    assert x.shape == (B, NU, IC, US), x.shape

    nc = _get_nc()
    xs = x.reshape(N_CORES, PB, HALVES, 128, F)
    w = np.zeros((128, NBANK, NBANK), dtype=np.float32)
    for k in range(NBANK):
        w[:, k, k] = 1.0 / IC
    in_maps = [{"x": np.ascontiguousarray(xs[c]), "w": w}
               for c in range(N_CORES)]
    res = run_bass_kernel_spmd(nc, in_maps, core_ids=list(range(N_CORES)),
                               **run_kwargs)
    # y is (NBANK, PB, 512) per core; batch-major reshape on the host.
    out = np.stack([r["y"].transpose(1, 0, 2) for r in res.results], axis=0)
    out = out.reshape(B, NU, US, 1)
    if run_kwargs:
        kernel.last_results = res
    return out
